# revision 1
# baseline (speedup 1.0000x reference)
"""Trainium2 Bass kernel for nn_CANE: data-parallel over batch on 8 NeuronCores.

Strategy: shard the batch (512 -> 64 items/core). Embedding tables (bf16,
rows padded to 128 elems = 256B) are replicated to every core's DRAM and
gathered on-device via transpose-mode dma_gather (text vocab split into two
<=32768-row halves to fit int16 indices; misses redirect to an all-zero row 0
and the two half-gathers are summed). All matmuls run in bf16 with fp32 PSUM
accumulation. Per-core scalar partial losses are summed on the host.
"""

import numpy as np
import ml_dtypes

import concourse.bass as bass
import concourse.bacc as bacc
import concourse.mybir as mybir
from concourse.tile import TileContext
from concourse import bass_utils

bf16 = ml_dtypes.bfloat16
F32 = mybir.dt.float32
BF = mybir.dt.bfloat16
I16 = mybir.dt.int16

B, NCORES = 512, 8
BL = B // NCORES            # 64 items per core
L, LM = 300, 299
E, C, V, NN = 100, 100, 50000, 100000
EP = 128                    # padded embedding row (256B in bf16)
NTOK = BL * L               # 19200 text tokens per tensor per core
TCH = 3200                  # gather chunk (25*128)
NCH = NTOK // TCH
HALF = 32767                # vocab ids < HALF go to the low table
NIDX = 256                  # node gather size (192 used, padded)
AF = mybir.ActivationFunctionType
ALU = mybir.AluOpType
AXL = mybir.AxisListType

# text tables: lo rows = 1 + HALF, hi rows = 1 + (V - HALF)
TLO_R, THI_R = HALF + 1, V - HALF + 1
# node tables: 4 splits of HALF ids each
NSPL = [(0, HALF), (HALF, 2 * HALF), (2 * HALF, 3 * HALF), (3 * HALF, NN)]
NTAB_R = [hi - lo + 1 for lo, hi in NSPL]

L_CK = [(0, 128), (128, 256), (256, 299)]   # l-chunks (the last is 43 wide)


def _wrap_idx(flat):
    """int16 flat index list -> [128, n/16] wrapped (i%16, i//16), x8 replicated."""
    n = flat.shape[0]
    assert n % 16 == 0
    w = flat.reshape(n // 16, 16).T.astype(np.int16)      # [16, n/16]
    return np.tile(w, (8, 1))                              # [128, n/16]


def _pad_rows(tab_f32):
    out = np.zeros((tab_f32.shape[0] + 1, EP), dtype=bf16)
    out[1:, :E] = tab_f32.astype(bf16)
    return out


def build_bass():
    nc = bacc.Bacc("TRN2", target_bir_lowering=False, debug=False)

    tlo = nc.dram_tensor("tlo", [TLO_R, EP], BF, kind="ExternalInput")
    thi = nc.dram_tensor("thi", [THI_R, EP], BF, kind="ExternalInput")
    ntab = [nc.dram_tensor(f"ntab{k}", [NTAB_R[k], EP], BF, kind="ExternalInput")
            for k in range(4)]
    tidx = nc.dram_tensor("tidx", [6, 128, NTOK // 16], I16, kind="ExternalInput")
    nidx = nc.dram_tensor("nidx", [4, 128, NIDX // 16], I16, kind="ExternalInput")
    w0td = nc.dram_tensor("w0td", [EP, C], BF, kind="ExternalInput")
    w1td = nc.dram_tensor("w1td", [EP, C], BF, kind="ExternalInput")
    rmatd = nc.dram_tensor("rmatd", [C, C], BF, kind="ExternalInput")
    biasd = nc.dram_tensor("biasd", [C, 1], F32, kind="ExternalInput")
    onesd = nc.dram_tensor("onesd", [128, 128], BF, kind="ExternalInput")  # all-ones
    identd = nc.dram_tensor("identd", [128, 128], BF, kind="ExternalInput")
    lossd = nc.dram_tensor("loss_out", [1, 1], F32, kind="ExternalOutput")

    with TileContext(nc) as tc:
        _emit(nc, tc, tlo, thi, ntab, tidx, nidx, w0td, w1td, rmatd, biasd,
              onesd, identd, lossd)
    nc.compile()  # Bacc: split multi-waits, insert library/act-table loads, lower ISA
    return nc


def _emit(nc, tc, tlo, thi, ntab, tidx, nidx, w0td, w1td, rmatd, biasd,
          onesd, identd, lossd):
    import contextlib
    ctx = contextlib.ExitStack()
    with ctx:
        const_p = ctx.enter_context(tc.tile_pool(name="const", bufs=1))
        txt_p = ctx.enter_context(tc.tile_pool(name="txt", bufs=1))
        raw_p = ctx.enter_context(tc.tile_pool(name="raw", bufs=2))
        work_p = ctx.enter_context(tc.tile_pool(name="work", bufs=3))
        coll_p = ctx.enter_context(tc.tile_pool(name="coll", bufs=1))
        bigps_p = ctx.enter_context(tc.tile_pool(name="bigps", bufs=1, space="PSUM"))
        smps_p = ctx.enter_context(tc.tile_pool(name="smps", bufs=2, space="PSUM"))

        # ---- constants into SBUF ----
        w0t = const_p.tile([EP, C], BF, name="w0t")
        w1t = const_p.tile([EP, C], BF, name="w1t")
        rmat = const_p.tile([C, C], BF, name="rmat")
        biasb = const_p.tile([C, 1], F32, name="biasb")
        onesb = const_p.tile([128, 128], BF, name="onesb")
        identb = const_p.tile([128, 128], BF, name="identb")
        nc.sync.dma_start(out=w0t[:, :], in_=w0td.ap())
        nc.sync.dma_start(out=w1t[:, :], in_=w1td.ap())
        nc.sync.dma_start(out=rmat[:, :], in_=rmatd.ap())
        nc.sync.dma_start(out=biasb[:, :], in_=biasd.ap())
        nc.sync.dma_start(out=onesb[:, :], in_=onesd.ap())
        nc.sync.dma_start(out=identb[:, :], in_=identd.ap())

        # ---- index tiles ----
        tix = const_p.tile([128, 6 * (NTOK // 16)], I16, name="tix")
        nix = const_p.tile([128, 4 * (NIDX // 16)], I16, name="nix")
        for t in range(6):
            nc.sync.dma_start(out=tix[:, t * (NTOK // 16):(t + 1) * (NTOK // 16)],
                              in_=tidx.ap()[t])
        for k in range(4):
            nc.sync.dma_start(out=nix[:, k * (NIDX // 16):(k + 1) * (NIDX // 16)],
                              in_=nidx.ap()[k])

        # ---- node gather: 4 splits summed; cols 3b+{0,1,2} = nA,nB,nN ----
        node_sb = coll_p.tile([128, NIDX], BF, name="node_sb")
        nraws = []
        for k in range(4):
            nraw = raw_p.tile([128, 1, NIDX], BF, name=f"nraw{k}", tag=f"nraw{k % 2}")
            nc.gpsimd.dma_gather(
                out_ap=nraw[:, :, :], in_ap=ntab[k].ap(),
                idxs_ap=nix[:, k * (NIDX // 16):(k + 1) * (NIDX // 16)],
                num_idxs=NIDX, num_idxs_reg=NIDX, elem_size=EP, transpose=True)
            nraws.append(nraw)
        # one DMA-wait per DVE op (multi-wait TT structs fail walrus codegen)
        nc.vector.tensor_copy(node_sb[:, :], nraws[0][:, 0, :])
        for k in (1, 2, 3):
            nc.vector.tensor_add(node_sb[:, :], node_sb[:, :], nraws[k][:, 0, :])

        # ---- text gathers: per tensor, 2 halves x NCH chunks, summed ----
        txts = []
        for t, tname in enumerate(("A", "B", "N")):
            txt = txt_p.tile([128, NTOK], BF, name=f"txt{tname}")
            txts.append(txt)
            for c in range(NCH):
                rhi = raw_p.tile([128, 1, TCH], BF, name=f"rhi{t}_{c}", tag="rhi")
                i0 = (2 * t) * (NTOK // 16) + c * (TCH // 16)
                i1 = (2 * t + 1) * (NTOK // 16) + c * (TCH // 16)
                dst = txt[:, c * TCH:(c + 1) * TCH]
                dst3 = txt.rearrange("p (k n) -> p k n", n=TCH)[:, c:c + 1, :]
                nc.gpsimd.dma_gather(
                    out_ap=dst3, in_ap=tlo.ap(),
                    idxs_ap=tix[:, i0:i0 + TCH // 16],
                    num_idxs=TCH, num_idxs_reg=TCH, elem_size=EP, transpose=True,
                    single_packet=False)
                nc.gpsimd.dma_gather(
                    out_ap=rhi[:, :, :], in_ap=thi.ap(),
                    idxs_ap=tix[:, i1:i1 + TCH // 16],
                    num_idxs=TCH, num_idxs_reg=TCH, elem_size=EP, transpose=True,
                    single_packet=False)
                nc.vector.tensor_add(dst, dst, rhi[:, 0, :])

        # ---- per-core collectors ----
        convcols = coll_p.tile([101, 3 * BL], F32, name="convcols")
        rawdots = coll_p.tile([1, 8 * BL], F32, name="rawdots")

        # ---- per-item pipeline ----
        for b in range(BL):
            cb = b * L
            bigp = bigps_p.tile([128, 6, 512], F32, name=f"bigp{b}", tag="bigp")
            hmrp = smps_p.tile([128, 512], F32, name=f"hmrp{b}", tag="smps")
            rowp = smps_p.tile([128, 512], F32, name=f"rowp{b}", tag="smps")
            bcpA = smps_p.tile([128, 512], F32, name=f"bcpA{b}", tag="smps")
            bcpB = smps_p.tile([128, 512], F32, name=f"bcpB{b}", tag="smps")
            bcpN = smps_p.tile([128, 512], F32, name=f"bcpN{b}", tag="smps")
            hx = work_p.tile([128, 3, LM], BF, name=f"hx{b}", tag="hx")
            hmrq = work_p.tile([C, 384], BF, name=f"hmrq{b}", tag="hmrq")
            t1 = work_p.tile([128, 6, LM], BF, name=f"t1_{b}", tag="t1")
            scr = work_p.tile([101, LM], BF, name=f"scr{b}", tag="scr")
            wraw = work_p.tile([128, 3], F32, name=f"wraw{b}", tag="wraw")
            eac = work_p.tile([128, 3], BF, name=f"eac{b}", tag="eac")
            erow = work_p.tile([1, 3, LM], BF, name=f"erow{b}", tag="erow")

            # conv: psum[0:100, t, 0:299] = w0t.T@txt[:,cb:cb+299] + w1t.T@(shift)
            for t in range(3):
                nc.tensor.matmul(bigp[0:C, t, 0:LM], w0t[:, :],
                                 txts[t][:, cb:cb + LM], start=True, stop=False)
            for t in range(3):
                nc.tensor.matmul(bigp[0:C, t, 0:LM], w1t[:, :],
                                 txts[t][:, cb + 1:cb + L], start=False, stop=True)
            # ones rows 96:128 first; conv-tanh then overwrites 96:100 with real
            # values, leaving rows 100+ = 1.0 (engine APs must start at 0/32/64/96)
            nc.vector.memset(hx[96:128, :, :], 1.0)
            nc.scalar.activation(hx[0:C, :, :], bigp[0:C, 0:3, 0:LM], AF.Tanh,
                                 bias=biasb[:, :], scale=1.0)

            # hmr: psum = rmat.T @ hAT ; copy to bf16, zero-pad cols 299:384
            nc.tensor.matmul(hmrp[0:C, 0:LM], rmat[:, :], hx[0:C, 0, :],
                             start=True, stop=True)
            nc.vector.tensor_copy(hmrq[:, 0:LM], hmrp[0:C, 0:LM])
            nc.vector.memset(hmrq[:, LM:384], 0.0)

            # att: slots 0-2 = att1 (rhs hB), slots 3-5 = att3 (rhs hN)
            for ck in range(3):
                lhs = hmrq[:, ck * 128:(ck + 1) * 128]
                nc.tensor.matmul(bigp[:, ck, 0:LM], lhs, hx[0:C, 1, :],
                                 start=True, stop=True)
                nc.tensor.matmul(bigp[:, 3 + ck, 0:LM], lhs, hx[0:C, 2, :],
                                 start=True, stop=True)
            nc.scalar.activation(t1[:, :, :], bigp[:, 0:6, 0:LM], AF.Tanh)

            # wA: free-dim reduce of att1 chunks -> [128,3]; exp -> bf16 cols
            nc.vector.tensor_reduce(wraw[:, :], t1[:, 0:3, :], axis=AXL.X, op=ALU.add)
            nc.scalar.activation(eac[:, :], wraw[:, :], AF.Exp, scale=1.0 / LM)

            # wB / wNEG: column sums via ones-matmuls (accumulate over chunks)
            for ck, (l0, l1) in enumerate(L_CK):
                w = l1 - l0
                nc.tensor.matmul(rowp[0:1, 0:LM], onesb[0:w, 0:1],
                                 t1[0:w, ck, :], start=(ck == 0), stop=(ck == 2))
                nc.tensor.matmul(hmrp[0:1, 0:LM], onesb[0:w, 0:1],
                                 t1[0:w, 3 + ck, :], start=(ck == 0), stop=(ck == 2))
            # rows: eB, eN from psum; eA via transpose of eac columns
            nc.scalar.activation(erow[:, 1, :], rowp[0:1, 0:LM], AF.Exp,
                                 scale=1.0 / LM)
            nc.scalar.activation(erow[:, 2, :], hmrp[0:1, 0:LM], AF.Exp,
                                 scale=1.0 / LM)
            for ck, (l0, l1) in enumerate(L_CK):
                w = l1 - l0
                nc.tensor.matmul(rowp[0:1, l0:l1], eac[0:w, ck:ck + 1],
                                 identb[0:w, 0:w], start=True, stop=True)
            nc.vector.tensor_copy(erow[:, 0, :], rowp[0:1, 0:LM])

            # broadcast rows to 101 partitions (outer product with ones col)
            bcps = (bcpA, bcpB, bcpN)
            for t in range(3):
                nc.tensor.matmul(bcps[t][0:C + 1, 0:LM], onesb[0:1, 0:C + 1],
                                 erow[:, t, :], start=True, stop=True)
            # conv vectors + sums: reduce of hx_ext * bc  (row 100 = ones -> sX)
            # (tensor_tensor_reduce hard-crashes this runtime; use mult+reduce)
            for t in range(3):
                nc.vector.tensor_tensor(out=scr[:, :], in0=hx[0:C + 1, t, :],
                                        in1=bcps[t][0:C + 1, 0:LM], op=ALU.mult)
                nc.vector.tensor_reduce(convcols[:, 3 * b + t:3 * b + t + 1],
                                        scr[:, :], axis=AXL.X, op=ALU.add)

        # ---- dots phase ----
        ccb = coll_p.tile([101, 3 * BL], BF, name="ccb")
        nc.vector.tensor_copy(ccb[:, :], convcols[:, :])
        for b in range(BL):
            dps = smps_p.tile([128, 512], F32, name=f"dps{b}", tag="smps")
            cA = ccb[0:C, 3 * b:3 * b + 1]
            cBN = ccb[0:C, 3 * b + 1:3 * b + 3]
            nA = node_sb[0:C, 3 * b:3 * b + 1]
            nBc = node_sb[0:C, 3 * b + 1:3 * b + 2]
            nBN = node_sb[0:C, 3 * b + 1:3 * b + 3]
            nc.tensor.matmul(dps[0:1, 0:2], cA, cBN, start=True, stop=True)
            nc.tensor.matmul(dps[0:1, 2:3], cA, nBc, start=True, stop=True)
            nc.tensor.matmul(dps[0:1, 3:5], nA, nBN, start=True, stop=True)
            nc.tensor.matmul(dps[0:1, 5:7], nA, cBN, start=True, stop=True)
            nc.tensor.matmul(dps[0:1, 7:8], nBc, ccb[0:C, 3 * b + 2:3 * b + 3],
                             start=True, stop=True)
            nc.vector.tensor_copy(rawdots[:, 8 * b:8 * b + 8], dps[0:1, 0:8])

        # ---- finals (row layout, vectorized over the 64 items) ----
        srow = coll_p.tile([1, 3 * BL], F32, name="srow")
        nc.sync.dma_start(out=srow[:, :], in_=convcols[C:C + 1, :])
        rr = coll_p.tile([1, 3 * BL], F32, name="rr")
        nc.vector.reciprocal(rr[:, :], srow[:, :])
        xs = coll_p.tile([1, 8 * BL], F32, name="xs")
        tmpa = coll_p.tile([1, BL], F32, name="tmpa")
        tmpb = coll_p.tile([1, BL], F32, name="tmpb")

        def dslice(k):
            return rawdots[0:1, k::8]

        def xslice(k):
            return xs[0:1, k::8]

        def rA():
            return rr[0:1, 0::3]

        def rB():
            return rr[0:1, 1::3]

        def rN():
            return rr[0:1, 2::3]

        # rawdots col order: [s1, s2, s7, s3, s4, s5, s6, s8]
        nc.vector.tensor_mul(tmpa[:, :], dslice(0), rA())
        nc.vector.tensor_mul(xslice(0), tmpa[:, :], rB())          # +s1 rA rB
        nc.vector.tensor_mul(tmpa[:, :], dslice(1), rA())
        nc.vector.tensor_mul(tmpb[:, :], tmpa[:, :], rN())
        nc.vector.tensor_scalar_mul(xslice(1), tmpb[:, :], -1.0)   # -s2 rA rN
        nc.vector.tensor_copy(xslice(2), dslice(3))                # +s3
        nc.vector.tensor_scalar_mul(xslice(3), dslice(4), -1.0)    # -s4
        nc.vector.tensor_mul(xslice(4), dslice(5), rB())           # +s5 rB
        nc.vector.tensor_mul(tmpa[:, :], dslice(6), rN())
        nc.vector.tensor_scalar_mul(xslice(5), tmpa[:, :], -1.0)   # -s6 rN
        nc.vector.tensor_mul(xslice(6), dslice(2), rA())           # +s7 rA
        nc.vector.tensor_mul(tmpa[:, :], dslice(7), rN())
        nc.vector.tensor_scalar_mul(xslice(7), tmpa[:, :], -1.0)   # -s8 rN

        sg = coll_p.tile([1, 8 * BL], F32, name="sg")
        pl = coll_p.tile([1, 8 * BL], F32, name="pl")
        nc.scalar.activation(sg[:, :], xs[:, :], AF.Sigmoid)
        nc.vector.tensor_scalar_add(sg[:, :], sg[:, :], 0.001)
        nc.scalar.activation(pl[:, :], sg[:, :], AF.Ln)

        def pslice(k):
            return pl[0:1, k::8]

        acc1 = coll_p.tile([1, BL], F32, name="acc1")
        acc3 = coll_p.tile([1, BL], F32, name="acc3")
        nc.vector.tensor_add(acc1[:, :], pslice(0), pslice(1))
        nc.vector.tensor_add(acc3[:, :], pslice(2), pslice(3))
        for k in (4, 5, 6, 7):
            nc.vector.tensor_add(acc3[:, :], acc3[:, :], pslice(k))
        nc.vector.tensor_scalar_mul(acc3[:, :], acc3[:, :], 0.3)
        nc.vector.tensor_add(acc1[:, :], acc1[:, :], acc3[:, :])
        lsum = coll_p.tile([1, 1], F32, name="lsum")
        nc.vector.tensor_reduce(lsum[:, :], acc1[:, :], axis=AXL.X, op=ALU.add)
        nc.vector.tensor_scalar_mul(lsum[:, :], lsum[:, :], -1.0)
        nc.sync.dma_start(out=lossd.ap(), in_=lsum[:, :])


# ----------------------------------------------------------------------------
# host side
# ----------------------------------------------------------------------------

def _text_idx_arrays(T):
    """T: [BL, L] int -> (lo, hi) wrapped int16 [128, NTOK/16]."""
    flat = T.reshape(-1).astype(np.int64)
    lo = np.where(flat < HALF, flat + 1, 0).astype(np.int16)
    hi = np.where(flat >= HALF, flat - HALF + 1, 0).astype(np.int16)
    return _wrap_idx(lo), _wrap_idx(hi)


def _node_idx_arrays(Na, Nb, Nn):
    inter = np.stack([Na, Nb, Nn], axis=1).reshape(-1).astype(np.int64)  # [192]
    inter = np.concatenate([inter, np.full(NIDX - inter.shape[0], -10, np.int64)])
    outs = []
    for lo, hi in NSPL:
        sel = (inter >= lo) & (inter < hi)
        ids = np.where(sel, inter - lo + 1, 0).astype(np.int16)
        outs.append(_wrap_idx(ids))
    return outs


_CACHED_NC = None


def kernel(**inputs):
    global _CACHED_NC
    text_emb = np.asarray(inputs["text_emb"], np.float32)
    node_emb = np.asarray(inputs["node_emb"], np.float32)
    conv_w = np.asarray(inputs["conv_w"], np.float32)
    conv_b = np.asarray(inputs["conv_b"], np.float32)
    rmat = np.asarray(inputs["rand_matrix"], np.float32)

    tlo_a = _pad_rows(text_emb[:HALF])                   # [32768, 128]
    thi_a = _pad_rows(text_emb[HALF:])
    ntab_a = [_pad_rows(node_emb[lo:hi]) for lo, hi in NSPL]
    w0t_a = np.zeros((EP, C), bf16); w0t_a[:E] = conv_w[:, 0, 0, :].T.astype(bf16)
    w1t_a = np.zeros((EP, C), bf16); w1t_a[:E] = conv_w[:, 0, 1, :].T.astype(bf16)
    rmat_a = rmat.astype(bf16)
    bias_a = conv_b.reshape(C, 1).astype(np.float32)
    ones_a = np.ones((128, 128), bf16)
    ident_a = np.eye(128, dtype=bf16)

    if _CACHED_NC is None:
        _CACHED_NC = build_bass()
    nc = _CACHED_NC

    in_maps = []
    for core in range(NCORES):
        sl = slice(core * BL, (core + 1) * BL)
        tA = np.asarray(inputs["Text_a"])[sl]
        tB = np.asarray(inputs["Text_b"])[sl]
        tN = np.asarray(inputs["Text_neg"])[sl]
        nA = np.asarray(inputs["Node_a"])[sl]
        nB = np.asarray(inputs["Node_b"])[sl]
        nN = np.asarray(inputs["Node_neg"])[sl]
        tidx_a = np.stack([w for T in (tA, tB, tN) for w in _text_idx_arrays(T)])
        nidx_a = np.stack(_node_idx_arrays(nA, nB, nN))
        m = {
            "tlo": tlo_a, "thi": thi_a,
            "tidx": tidx_a, "nidx": nidx_a,
            "w0td": w0t_a, "w1td": w1t_a, "rmatd": rmat_a, "biasd": bias_a,
            "onesd": ones_a, "identd": ident_a,
        }
        for k in range(4):
            m[f"ntab{k}"] = ntab_a[k]
        in_maps.append(m)

    res = bass_utils.run_bass_kernel_spmd(nc, in_maps, core_ids=list(range(NCORES)))
    parts = [float(r["loss_out"][0, 0]) for r in res.results]
    return np.float32(np.sum(parts, dtype=np.float64))



# revision 2
# speedup vs baseline: 1.1691x; 1.1691x over previous
"""Trainium2 Bass kernel for nn_CANE v2: data-parallel over batch on 8 cores.

Differences from v1: text gathers are interleaved (chunk-triples across the
three text tensors, emitted just-in-time inside the item loop) so the per-item
pipeline starts ~25us in; the conv-vector contractions run on PE via
identity-transposed tanh tiles and 1-col accumulating matmuls into a
persistent PSUM collector (no per-item DVE mult/reduce/broadcast); the
softmax weight sums are produced directly in column layout by tiny
ones-matmuls and a single merged exp per item; copies are split between
Pool and DVE to keep the Activation engine the only near-saturated engine.
"""

import numpy as np
import ml_dtypes

import concourse.bass as bass
import concourse.bacc as bacc
import concourse.mybir as mybir
from concourse.tile import TileContext
from concourse import bass_utils

bf16 = ml_dtypes.bfloat16
F32 = mybir.dt.float32
BF = mybir.dt.bfloat16
I16 = mybir.dt.int16

B, NCORES = 512, 8
BL = B // NCORES            # 64 items per core
L, LM = 300, 299
E, C, V, NN = 100, 100, 50000, 100000
EP = 128                    # padded embedding row (256B in bf16)
NTOK = BL * L               # 19200 text tokens per tensor per core
TCH = 3200                  # gather chunk (25*128)
NCH = NTOK // TCH           # 6 chunks
HALF = 32767
NIDX = 256
AF = mybir.ActivationFunctionType
ALU = mybir.AluOpType
AXL = mybir.AxisListType

TLO_R, THI_R = HALF + 1, V - HALF + 1
NSPL = [(0, HALF), (HALF, 2 * HALF), (2 * HALF, 3 * HALF), (3 * HALF, NN)]
NTAB_R = [hi - lo + 1 for lo, hi in NSPL]

L_CK = [(0, 128), (128, 256), (256, 299)]   # l/m chunks (last is 43 wide)

# triple c is emitted before item TRIPLE_AT[c]; node gathers at item 50
TRIPLE_AT = {2: 6, 3: 17, 4: 27, 5: 38}
NODE_AT = 50


def _wrap_idx(flat):
    n = flat.shape[0]
    assert n % 16 == 0
    w = flat.reshape(n // 16, 16).T.astype(np.int16)
    return np.tile(w, (8, 1))


def _pad_rows(tab_f32):
    out = np.zeros((tab_f32.shape[0] + 1, EP), dtype=bf16)
    out[1:, :E] = tab_f32.astype(bf16)
    return out


def build_bass():
    nc = bacc.Bacc("TRN2", target_bir_lowering=False, debug=False)

    tlo = nc.dram_tensor("tlo", [TLO_R, EP], BF, kind="ExternalInput")
    thi = nc.dram_tensor("thi", [THI_R, EP], BF, kind="ExternalInput")
    ntab = [nc.dram_tensor(f"ntab{k}", [NTAB_R[k], EP], BF, kind="ExternalInput")
            for k in range(4)]
    tidx = nc.dram_tensor("tidx", [6, 128, NTOK // 16], I16, kind="ExternalInput")
    nidx = nc.dram_tensor("nidx", [4, 128, NIDX // 16], I16, kind="ExternalInput")
    w0td = nc.dram_tensor("w0td", [EP, C], BF, kind="ExternalInput")
    w1td = nc.dram_tensor("w1td", [EP, C], BF, kind="ExternalInput")
    rmatd = nc.dram_tensor("rmatd", [C, C], BF, kind="ExternalInput")
    biasd = nc.dram_tensor("biasd", [C, 1], F32, kind="ExternalInput")
    onesd = nc.dram_tensor("onesd", [128, 128], BF, kind="ExternalInput")
    identd = nc.dram_tensor("identd", [128, 128], BF, kind="ExternalInput")
    lossd = nc.dram_tensor("loss_out", [1, 1], F32, kind="ExternalOutput")

    with TileContext(nc) as tc:
        _emit(nc, tc, tlo, thi, ntab, tidx, nidx, w0td, w1td, rmatd, biasd,
              onesd, identd, lossd)
    nc.compile()
    return nc


def _emit(nc, tc, tlo, thi, ntab, tidx, nidx, w0td, w1td, rmatd, biasd,
          onesd, identd, lossd):
    import contextlib
    ctx = contextlib.ExitStack()
    with ctx:
        const_p = ctx.enter_context(tc.tile_pool(name="const", bufs=1))
        txt_p = ctx.enter_context(tc.tile_pool(name="txt", bufs=1))
        raw_p = ctx.enter_context(tc.tile_pool(name="raw", bufs=3))
        work_p = ctx.enter_context(tc.tile_pool(name="work", bufs=1))
        coll_p = ctx.enter_context(tc.tile_pool(name="coll", bufs=1))
        psA_p = ctx.enter_context(tc.tile_pool(name="psA", bufs=1, space="PSUM"))
        psB_p = ctx.enter_context(tc.tile_pool(name="psB", bufs=1, space="PSUM"))
        psC_p = ctx.enter_context(tc.tile_pool(name="psC", bufs=1, space="PSUM"))
        psD_p = ctx.enter_context(tc.tile_pool(name="psD", bufs=1, space="PSUM"))
        hx_p = ctx.enter_context(tc.tile_pool(name="hxp", bufs=3))
        hm_p = ctx.enter_context(tc.tile_pool(name="hmp", bufs=2))
        t1_p = ctx.enter_context(tc.tile_pool(name="t1p", bufs=2))
        hq_p = ctx.enter_context(tc.tile_pool(name="hqp", bufs=2))
        ea_p = ctx.enter_context(tc.tile_pool(name="eap", bufs=2))

        # ---- constants ----
        w0t = const_p.tile([EP, C], BF, name="w0t")
        w1t = const_p.tile([EP, C], BF, name="w1t")
        rmat = const_p.tile([C, C], BF, name="rmat")
        biasb = const_p.tile([C, 1], F32, name="biasb")
        onesb = const_p.tile([128, 128], BF, name="onesb")
        identb = const_p.tile([128, 128], BF, name="identb")
        nc.sync.dma_start(out=w0t[:, :], in_=w0td.ap())
        nc.sync.dma_start(out=w1t[:, :], in_=w1td.ap())
        nc.sync.dma_start(out=rmat[:, :], in_=rmatd.ap())
        nc.sync.dma_start(out=biasb[:, :], in_=biasd.ap())
        nc.sync.dma_start(out=onesb[:, :], in_=onesd.ap())
        nc.sync.dma_start(out=identb[:, :], in_=identd.ap())

        # ---- index tiles ----
        tix = const_p.tile([128, 6 * (NTOK // 16)], I16, name="tix")
        nix = const_p.tile([128, 4 * (NIDX // 16)], I16, name="nix")
        for t in range(6):
            nc.sync.dma_start(out=tix[:, t * (NTOK // 16):(t + 1) * (NTOK // 16)],
                              in_=tidx.ap()[t])
        for k in range(4):
            nc.sync.dma_start(out=nix[:, k * (NIDX // 16):(k + 1) * (NIDX // 16)],
                              in_=nidx.ap()[k])

        # ---- text tiles + gather emitters ----
        txts = [txt_p.tile([128, NTOK], BF, name=f"txt{n}") for n in "ABN"]

        def emit_triple(c):
            for t in range(3):
                rhi = raw_p.tile([128, 1, TCH], BF, name=f"rhi{t}_{c}", tag="rhi")
                i0 = (2 * t) * (NTOK // 16) + c * (TCH // 16)
                i1 = (2 * t + 1) * (NTOK // 16) + c * (TCH // 16)
                dst = txts[t][:, c * TCH:(c + 1) * TCH]
                dst3 = txts[t].rearrange("p (k n) -> p k n", n=TCH)[:, c:c + 1, :]
                nc.gpsimd.dma_gather(
                    out_ap=dst3, in_ap=tlo.ap(),
                    idxs_ap=tix[:, i0:i0 + TCH // 16],
                    num_idxs=TCH, num_idxs_reg=TCH, elem_size=EP, transpose=True,
                    single_packet=False)
                nc.gpsimd.dma_gather(
                    out_ap=rhi[:, :, :], in_ap=thi.ap(),
                    idxs_ap=tix[:, i1:i1 + TCH // 16],
                    num_idxs=TCH, num_idxs_reg=TCH, elem_size=EP, transpose=True,
                    single_packet=False)
                nc.vector.tensor_add(dst, dst, rhi[:, 0, :])

        node_sb = coll_p.tile([128, NIDX], BF, name="node_sb")
        nraws = []

        def emit_node_gathers():
            for k in range(4):
                nraw = raw_p.tile([128, 1, NIDX], BF, name=f"nraw{k}",
                                  tag=f"nraw{k % 2}")
                nc.gpsimd.dma_gather(
                    out_ap=nraw[:, :, :], in_ap=ntab[k].ap(),
                    idxs_ap=nix[:, k * (NIDX // 16):(k + 1) * (NIDX // 16)],
                    num_idxs=NIDX, num_idxs_reg=NIDX, elem_size=EP, transpose=True)
                nraws.append(nraw)

        emit_triple(0)
        emit_triple(1)

        # ---- persistent PSUM banks ----
        Pconv = psA_p.tile([128, 3, 512], F32, name="Pconv")  # conv 0:299 | hxtA 304:404 | hxtB 404:504
        Patt = psB_p.tile([128, 3, 512], F32, name="Patt")    # att 0:299 | bank0 300:492 = s-collector
        Pmisc = psC_p.tile([128, 512], F32, name="Pmisc")     # hmr 0:299 | wraw/e 384:393
        Pcoll = psD_p.tile([128, 512], F32, name="Pcoll")     # hxtN 0:300 | collector 300:492

        # ---- software-pipelined per-item loop ----
        # Period b emits: pre(b+1) [conv/hmr/transposes], att1(b)+wA/wB(b),
        # exp(b-1), att3mm(b), convvec(b-1), att3tanh(b), wNEG(b).
        # e-columns double-buffer by item parity so exp(b-1) and wraw(b)
        # don't race on the same Pmisc region.
        S = {}

        def ecol0(j):
            return 384 if j % 2 == 0 else 400

        def stage_pre(j):
            cb = j * L
            hx = hx_p.tile([128, 3, LM], BF, name=f"hx{j}", tag="hx")
            hmrq = hm_p.tile([C, 384], BF, name=f"hmrq{j}", tag="hmrq")
            hqA = hq_p.tile([128, 300], BF, name=f"hqA{j}", tag="hqA")
            hqB = hq_p.tile([128, 300], BF, name=f"hqB{j}", tag="hqB")
            hqN = hq_p.tile([128, 300], BF, name=f"hqN{j}", tag="hqN")
            S[j] = dict(hx=hx, hmrq=hmrq, hq=(hqA, hqB, hqN))

            for t in range(3):
                nc.tensor.matmul(Pconv[0:C, t, 0:LM], w0t[:, :],
                                 txts[t][:, cb:cb + LM], start=True, stop=False)
            for t in range(3):
                nc.tensor.matmul(Pconv[0:C, t, 0:LM], w1t[:, :],
                                 txts[t][:, cb + 1:cb + L], start=False, stop=True)
            nc.scalar.activation(hx[0:C, :, :], Pconv[0:C, 0:3, 0:LM], AF.Tanh,
                                 bias=biasb[:, :], scale=1.0)

            nc.tensor.matmul(Pmisc[0:C, 0:LM], rmat[:, :], hx[0:C, 0, :],
                             start=True, stop=True)
            if j < 2:
                nc.vector.memset(hmrq[:, LM:384], 0.0)
            nc.vector.tensor_copy(hmrq[:, 0:LM], Pmisc[0:C, 0:LM])

            for ck, (l0, l1) in enumerate(L_CK):
                w = l1 - l0
                nc.tensor.matmul(Pconv[0:w, ck, 304:404], hx[0:C, 0, l0:l1],
                                 identb[0:C, 0:C], start=True, stop=True)
                nc.tensor.matmul(Pconv[0:w, ck, 404:504], hx[0:C, 1, l0:l1],
                                 identb[0:C, 0:C], start=True, stop=True)
                nc.tensor.matmul(Pcoll[0:w, 100 * ck:100 * ck + 100],
                                 hx[0:C, 2, l0:l1],
                                 identb[0:C, 0:C], start=True, stop=True)
            nc.vector.tensor_copy(
                hqA.rearrange("p (k n) -> p k n", n=100)[:, :, :],
                Pconv[0:128, 0:3, 304:404])
            nc.vector.tensor_copy(
                hqB.rearrange("p (k n) -> p k n", n=100)[:, :, :],
                Pconv[0:128, 0:3, 404:504])
            nc.vector.tensor_copy(hqN[:, :], Pcoll[0:128, 0:300])

        def stage_att1(j):
            hx, hmrq = S[j]["hx"], S[j]["hmrq"]
            t1 = t1_p.tile([128, 6, LM], BF, name=f"t1_{j}", tag="t1")
            S[j]["t1"] = t1
            e0 = ecol0(j)
            for ck in range(3):
                nc.tensor.matmul(Patt[:, ck, 0:LM],
                                 hmrq[:, ck * 128:(ck + 1) * 128],
                                 hx[0:C, 1, :], start=True, stop=True)
            nc.scalar.activation(t1[:, 0:3, :], Patt[:, 0:3, 0:LM], AF.Tanh)
            nc.vector.tensor_reduce(Pmisc[0:128, e0:e0 + 3], t1[:, 0:3, :],
                                    axis=AXL.X, op=ALU.add)
            for mck, (m0, m1) in enumerate(L_CK):
                wm = m1 - m0
                for ck, (l0, l1) in enumerate(L_CK):
                    wl = l1 - l0
                    nc.tensor.matmul(Pmisc[0:wm, e0 + 3 + mck:e0 + 4 + mck],
                                     t1[0:wl, ck, m0:m1], onesb[0:wl, 0:1],
                                     start=(ck == 0), stop=(ck == 2))

        def emit_exp(j):
            eall = ea_p.tile([128, 9], BF, name=f"eall{j}", tag="eall")
            S[j]["eall"] = eall
            e0 = ecol0(j)
            nc.scalar.activation(eall[:, :], Pmisc[0:128, e0:e0 + 9], AF.Exp,
                                 scale=1.0 / LM)

        def emit_att3_mm(j):
            hx, hmrq = S[j]["hx"], S[j]["hmrq"]
            for ck in range(3):
                nc.tensor.matmul(Patt[:, ck, 0:LM],
                                 hmrq[:, ck * 128:(ck + 1) * 128],
                                 hx[0:C, 2, :], start=True, stop=True)

        def emit_convvec(j):
            eall = S[j]["eall"]
            for t in range(3):
                hq = S[j]["hq"][t]
                col = 300 + 3 * j + t
                for ck, (l0, l1) in enumerate(L_CK):
                    wl = l1 - l0
                    nc.tensor.matmul(Pcoll[0:C, col:col + 1],
                                     hq[0:wl, 100 * ck:100 * ck + 100],
                                     eall[0:wl, 3 * t + ck:3 * t + ck + 1],
                                     start=(ck == 0), stop=(ck == 2))
                    nc.tensor.matmul(Patt[0:1, 0, col:col + 1],
                                     onesb[0:wl, 0:1],
                                     eall[0:wl, 3 * t + ck:3 * t + ck + 1],
                                     start=(ck == 0), stop=(ck == 2))
            del S[j]

        def emit_att3_tail(j):
            t1 = S[j]["t1"]
            e0 = ecol0(j)
            nc.scalar.activation(t1[:, 3:6, :], Patt[:, 0:3, 0:LM], AF.Tanh)
            for mck, (m0, m1) in enumerate(L_CK):
                wm = m1 - m0
                for ck, (l0, l1) in enumerate(L_CK):
                    wl = l1 - l0
                    nc.tensor.matmul(Pmisc[0:wm, e0 + 6 + mck:e0 + 7 + mck],
                                     t1[0:wl, 3 + ck, m0:m1], onesb[0:wl, 0:1],
                                     start=(ck == 0), stop=(ck == 2))

        stage_pre(0)
        for b in range(BL):
            for c, at in TRIPLE_AT.items():
                if b == at:
                    emit_triple(c)
            if b == NODE_AT:
                emit_node_gathers()
            if b + 1 < BL:
                stage_pre(b + 1)
            stage_att1(b)
            emit_att3_mm(b)
            emit_att3_tail(b)
            emit_exp(b)
            emit_convvec(b)

        # ---- node embedding sum ----
        nc.vector.tensor_copy(node_sb[:, :], nraws[0][:, 0, :])
        for k in (1, 2, 3):
            nc.vector.tensor_add(node_sb[:, :], node_sb[:, :], nraws[k][:, 0, :])

        # ---- dots phase ----
        rrt = coll_p.tile([1, 3 * BL], F32, name="rrt")
        nc.vector.reciprocal(rrt[:, :], Patt[0:1, 0, 300:300 + 3 * BL])

        dcols = coll_p.tile([128, 8 * BL], BF, name="dcols")
        cc0 = 300
        nc.vector.tensor_copy(dcols[0:C, 0::8], Pcoll[0:C, cc0 + 0:cc0 + 192:3])
        nc.vector.tensor_copy(dcols[0:C, 3::8], Pcoll[0:C, cc0 + 1:cc0 + 192:3])
        nc.vector.tensor_copy(dcols[0:C, 4::8], Pcoll[0:C, cc0 + 2:cc0 + 192:3])
        nc.vector.tensor_copy(dcols[0:C, 1::8], node_sb[0:C, 0:192:3])
        nc.vector.tensor_copy(dcols[0:C, 2::8], node_sb[0:C, 1:192:3])
        nc.vector.tensor_copy(dcols[0:C, 5::8], node_sb[0:C, 1:192:3])
        nc.vector.tensor_copy(dcols[0:C, 6::8], node_sb[0:C, 2:192:3])

        for b in range(BL):
            o = 8 * b
            nc.tensor.matmul(Pmisc[0:1, 4 * b:4 * b + 4],
                             dcols[0:C, o:o + 1], dcols[0:C, o + 3:o + 7],
                             start=True, stop=True)
            nc.tensor.matmul(Pmisc[0:1, 256 + 4 * b:256 + 4 * b + 4],
                             dcols[0:C, o + 1:o + 2], dcols[0:C, o + 3:o + 7],
                             start=True, stop=True)
            nc.tensor.matmul(Pconv[0:1, 0, b:b + 1],
                             dcols[0:C, o + 2:o + 3], dcols[0:C, o + 4:o + 5],
                             start=True, stop=True)
        g0 = coll_p.tile([1, 512], F32, name="g0")
        g2 = coll_p.tile([1, BL], F32, name="g2")
        nc.vector.tensor_copy(g0[:, :], Pmisc[0:1, 0:512])
        nc.vector.tensor_copy(g2[:, :], Pconv[0:1, 0, 0:BL])

        # ---- finals ----
        # g0 row: cA.[cB,cN,nB,nN] at 0:256, nA.[cB,cN,nB,nN] at 256:512
        s1 = g0[0:1, 0:256:4]
        s2 = g0[0:1, 1:256:4]
        s7 = g0[0:1, 2:256:4]
        s5 = g0[0:1, 256::4]
        s6 = g0[0:1, 257::4]
        s3 = g0[0:1, 258::4]
        s4 = g0[0:1, 259::4]
        s8 = g2[0:1, :]

        def rA():
            return rrt[0:1, 0::3]

        def rB():
            return rrt[0:1, 1::3]

        def rN():
            return rrt[0:1, 2::3]

        xs = coll_p.tile([1, 8 * BL], F32, name="xs")
        tmpa = coll_p.tile([1, BL], F32, name="tmpa")
        tmpb = coll_p.tile([1, BL], F32, name="tmpb")

        def xslice(k):
            return xs[0:1, k::8]

        nc.vector.tensor_mul(tmpa[:, :], s1, rA())
        nc.vector.tensor_mul(xslice(0), tmpa[:, :], rB())          # +s1 rA rB
        nc.vector.tensor_mul(tmpa[:, :], s2, rA())
        nc.vector.tensor_mul(tmpb[:, :], tmpa[:, :], rN())
        nc.vector.tensor_scalar_mul(xslice(1), tmpb[:, :], -1.0)   # -s2 rA rN
        nc.vector.tensor_copy(xslice(2), s3)                       # +s3
        nc.vector.tensor_scalar_mul(xslice(3), s4, -1.0)           # -s4
        nc.vector.tensor_mul(xslice(4), s5, rB())                  # +s5 rB
        nc.vector.tensor_mul(tmpa[:, :], s6, rN())
        nc.vector.tensor_scalar_mul(xslice(5), tmpa[:, :], -1.0)   # -s6 rN
        nc.vector.tensor_mul(xslice(6), s7, rA())                  # +s7 rA
        nc.vector.tensor_mul(tmpa[:, :], s8, rN())
        nc.vector.tensor_scalar_mul(xslice(7), tmpa[:, :], -1.0)   # -s8 rN

        sg = coll_p.tile([1, 8 * BL], F32, name="sg")
        pl = coll_p.tile([1, 8 * BL], F32, name="pl")
        nc.scalar.activation(sg[:, :], xs[:, :], AF.Sigmoid)
        nc.vector.tensor_scalar_add(sg[:, :], sg[:, :], 0.001)
        nc.scalar.activation(pl[:, :], sg[:, :], AF.Ln)

        def pslice(k):
            return pl[0:1, k::8]

        acc1 = coll_p.tile([1, BL], F32, name="acc1")
        acc3 = coll_p.tile([1, BL], F32, name="acc3")
        nc.vector.tensor_add(acc1[:, :], pslice(0), pslice(1))
        nc.vector.tensor_add(acc3[:, :], pslice(2), pslice(3))
        for k in (4, 5, 6, 7):
            nc.vector.tensor_add(acc3[:, :], acc3[:, :], pslice(k))
        nc.vector.tensor_scalar_mul(acc3[:, :], acc3[:, :], 0.3)
        nc.vector.tensor_add(acc1[:, :], acc1[:, :], acc3[:, :])
        lsum = coll_p.tile([1, 1], F32, name="lsum")
        nc.vector.tensor_reduce(lsum[:, :], acc1[:, :], axis=AXL.X, op=ALU.add)
        nc.vector.tensor_scalar_mul(lsum[:, :], lsum[:, :], -1.0)
        nc.sync.dma_start(out=lossd.ap(), in_=lsum[:, :])


# ----------------------------------------------------------------------------
# host side
# ----------------------------------------------------------------------------

def _text_idx_arrays(T):
    flat = T.reshape(-1).astype(np.int64)
    lo = np.where(flat < HALF, flat + 1, 0).astype(np.int16)
    hi = np.where(flat >= HALF, flat - HALF + 1, 0).astype(np.int16)
    return _wrap_idx(lo), _wrap_idx(hi)


def _node_idx_arrays(Na, Nb, Nn):
    inter = np.stack([Na, Nb, Nn], axis=1).reshape(-1).astype(np.int64)
    inter = np.concatenate([inter, np.full(NIDX - inter.shape[0], -10, np.int64)])
    outs = []
    for lo, hi in NSPL:
        sel = (inter >= lo) & (inter < hi)
        ids = np.where(sel, inter - lo + 1, 0).astype(np.int16)
        outs.append(_wrap_idx(ids))
    return outs


_CACHED_NC = None


def kernel(**inputs):
    global _CACHED_NC
    text_emb = np.asarray(inputs["text_emb"], np.float32)
    node_emb = np.asarray(inputs["node_emb"], np.float32)
    conv_w = np.asarray(inputs["conv_w"], np.float32)
    conv_b = np.asarray(inputs["conv_b"], np.float32)
    rmat = np.asarray(inputs["rand_matrix"], np.float32)

    tlo_a = _pad_rows(text_emb[:HALF])
    thi_a = _pad_rows(text_emb[HALF:])
    ntab_a = [_pad_rows(node_emb[lo:hi]) for lo, hi in NSPL]
    w0t_a = np.zeros((EP, C), bf16); w0t_a[:E] = conv_w[:, 0, 0, :].T.astype(bf16)
    w1t_a = np.zeros((EP, C), bf16); w1t_a[:E] = conv_w[:, 0, 1, :].T.astype(bf16)
    rmat_a = rmat.astype(bf16)
    bias_a = conv_b.reshape(C, 1).astype(np.float32)
    ones_a = np.ones((128, 128), bf16)
    ident_a = np.eye(128, dtype=bf16)

    if _CACHED_NC is None:
        _CACHED_NC = build_bass()
    nc = _CACHED_NC

    in_maps = []
    for core in range(NCORES):
        sl = slice(core * BL, (core + 1) * BL)
        tA = np.asarray(inputs["Text_a"])[sl]
        tB = np.asarray(inputs["Text_b"])[sl]
        tN = np.asarray(inputs["Text_neg"])[sl]
        nA = np.asarray(inputs["Node_a"])[sl]
        nB = np.asarray(inputs["Node_b"])[sl]
        nN = np.asarray(inputs["Node_neg"])[sl]
        tidx_a = np.stack([w for T in (tA, tB, tN) for w in _text_idx_arrays(T)])
        nidx_a = np.stack(_node_idx_arrays(nA, nB, nN))
        m = {
            "tlo": tlo_a, "thi": thi_a,
            "tidx": tidx_a, "nidx": nidx_a,
            "w0td": w0t_a, "w1td": w1t_a, "rmatd": rmat_a, "biasd": bias_a,
            "onesd": ones_a, "identd": ident_a,
        }
        for k in range(4):
            m[f"ntab{k}"] = ntab_a[k]
        in_maps.append(m)

    res = bass_utils.run_bass_kernel_spmd(nc, in_maps, core_ids=list(range(NCORES)))
    parts = [float(r["loss_out"][0, 0]) for r in res.results]
    return np.float32(np.sum(parts, dtype=np.float64))
